# revision 22
# baseline (speedup 1.0000x reference)
"""Trainium2 Bass kernel for nn_AdditiveAttention (B=8, S=4096, D=1024, H=16).

Sharding: pure data-parallel over batch — 8 NeuronCores, one batch element
per core, weights replicated. No collectives.

Per-core layout: everything transposed (d on partitions, s on free).

v3 structure:
  - Q phase is s-quarter-outer: each 2.1 MB quarter-column of xt is
    consumed as its DMAs land, so the PE starts ~12us into the kernel and
    streams from there. Warm-up matmuls bridge the DMA window (HAM
    un-throttle + ACT exp-table load).
  - All per-chunk ops are 1024-wide (two 512 psum banks per tile) to
    amortize the ~300ns fixed per-op engine overhead.
  - Pooling is incremental: ACT Exp writes accum_out Z-partials, a fused
    DVE scalar_tensor_tensor(mult, mult, accum_out) does the numerator.
    K-phase psum evacuations ride DVE (tensor_scalar_add) to keep ACT
    at the exp-only load.
  - The elementwise V gate u = (v+bv)*gk and the r-projection fold into
    the weights on-device: Wcomb = Wv @ diag(gk_h) @ Wr per head (64
    N=128 fp8 matmuls from Wv^T, built per head-pair block as that
    block's gk finalizes inside the K phase), so the V phase is a single
    streaming fp8 DoubleRow GEMM producing the rt output directly.
  - Outputs are bf16 (q-residual and rt part summed on host in f32).
"""

import sys
import types

import numpy as np
import ml_dtypes

from contextlib import ExitStack

import concourse.bass as bass
import concourse.tile as tile
from concourse import bacc, mybir
from concourse.bass_utils import run_bass_kernel_spmd

B, S, D, H, HD = 8, 4096, 1024, 16, 64
P = 128          # partitions
T = D // P       # 8 d-tiles
NC_ = 512        # psum bank free size
NP = 1024        # paired op width
NG = S // NP     # 4 quarter/pair groups
N_CORES = 8
BF16 = mybir.dt.bfloat16
FP8 = mybir.dt.float8e4
F32 = mybir.dt.float32
W8SCALE = 64.0   # host scales Wk/WvT/Wr by this into e4m3 normal range
GK8 = 256.0      # device scale for gk -> fp8 operand
WCE = 2.0 ** -4  # psWC -> wcomb8 evacuation scale
VDE = 2.0 ** -18 # V-phase psum -> output scale (undoes 64*64*256*WCE)
BF = ml_dtypes.bfloat16
F8 = ml_dtypes.float8_e4m3

_CACHE = {}


def _build():
    nc = bacc.Bacc(
        "TRN2", target_bir_lowering=False, debug=False, num_devices=N_CORES
    )
    xt_ext = nc.declare_dram_parameter("xt", [D, S], BF16, isOutput=False)
    wq_ext = nc.declare_dram_parameter("wq", [D, D], BF16, isOutput=False)
    wk_ext = nc.declare_dram_parameter("wk", [D, D], FP8, isOutput=False)
    wvt_ext = nc.declare_dram_parameter("wvt", [D, D], FP8, isOutput=False)
    bq_ext = nc.declare_dram_parameter("bq", [P, T], F32, isOutput=False)
    bk_ext = nc.declare_dram_parameter("bk", [P, T], F32, isOutput=False)
    bv_ext = nc.declare_dram_parameter("bv", [P, T], F32, isOutput=False)
    wql_ext = nc.declare_dram_parameter("wqlrep", [P, P], BF16, isOutput=False)
    wkl_ext = nc.declare_dram_parameter("wklrep", [P, P], BF16, isOutput=False)
    wrr_ext = nc.declare_dram_parameter("wrr", [P, P], BF16, isOutput=False)
    wrr8_ext = nc.declare_dram_parameter("wrr8", [P, P], FP8, isOutput=False)
    br_ext = nc.declare_dram_parameter("br", [P, 1], F32, isOutput=False)
    out_ext = nc.declare_dram_parameter("out", [D, S], BF16, isOutput=True)
    out2_ext = nc.declare_dram_parameter("out2", [D, S], BF16, isOutput=True)

    AX = mybir.AxisListType.X
    ALU = mybir.AluOpType
    AF = mybir.ActivationFunctionType

    with tile.TileContext(nc) as tc, ExitStack() as ctx:
        singles = ctx.enter_context(tc.tile_pool(name="singles", bufs=1))
        psum = ctx.enter_context(tc.tile_pool(name="psum", bufs=2, space="PSUM"))
        pslg = ctx.enter_context(tc.tile_pool(name="pslg", bufs=2, space="PSUM"))
        ch_pool = ctx.enter_context(tc.tile_pool(name="chpool", bufs=5))
        e_pool = ctx.enter_context(tc.tile_pool(name="epool", bufs=5))
        eff_pool = ctx.enter_context(tc.tile_pool(name="eff", bufs=2))
        small_pool = ctx.enter_context(tc.tile_pool(name="small", bufs=4))

        # ---- resident tiles ----
        xt_sb = singles.tile([P, T, S], BF16, name="xt", tag="xt")
        xt8_sb = singles.tile([P, T, S], FP8, name="xt8", tag="xt8")
        wq_sb = singles.tile([P, T, D], BF16, name="wq", tag="wq")
        wk_sb = singles.tile([P, T, D], FP8, name="wk", tag="wk")
        wvt_sb = singles.tile([P, T, D], FP8, name="wvt", tag="wvt")
        wcomb_sb = singles.tile([P, T, D], FP8, name="wcomb", tag="wcomb")
        wqlrep = singles.tile([P, P], BF16, name="wqlrep", tag="wqlrep")
        wklrep = singles.tile([P, P], BF16, name="wklrep", tag="wklrep")
        wrr = singles.tile([P, P], BF16, name="wrr", tag="wrr")
        wrr8 = singles.tile([P, P], FP8, name="wrr8", tag="wrr8")
        bq_sb = singles.tile([P, T], F32, name="bq", tag="bq")
        bk_sb = singles.tile([P, T], F32, name="bk", tag="bk")
        bk16_sb = singles.tile([P, T], BF16, name="bk16", tag="bk16")
        cb_sb = singles.tile([P, T], F32, name="cb", tag="cb")
        bv_sb = singles.tile([P, T], F32, name="bv", tag="bv")
        bvg_sb = singles.tile([P, T], BF16, name="bvg", tag="bvg")
        br_sb = singles.tile([P, 1], F32, name="br", tag="br")
        bias2_sb = singles.tile([P, T], F32, name="bias2", tag="bias2")
        zq_sb = singles.tile([P, T, NG], F32, name="zq", tag="zq")
        gqp_sb = singles.tile([P, T, NG], F32, name="gqp", tag="gqp")
        zk_sb = singles.tile([P, T, NG], F32, name="zk", tag="zk")
        gkp_sb = singles.tile([P, T, NG], F32, name="gkp", tag="gkp")
        gq_all = singles.tile([P, T], F32, name="gq", tag="gq")
        gk_all = singles.tile([P, T], F32, name="gk", tag="gk")
        gk8c = singles.tile([P, 1], F32, name="gk8c", tag="gk8c")
        effk_sb = singles.tile([P, T, P], BF16, name="effk", tag="effk")
        scratch = singles.tile([P, NC_], BF16, name="scr", tag="scr")
        scr_e = singles.tile([P, NC_], BF16, name="scre", tag="scre")

        # ---- DMA issue ----
        # sync queue: small weights, then wq in t-major blocks (each block
        # unlocks one output tile's GEMMs — block 0 lands ~5us).
        nc.sync.dma_start(wqlrep[:], wql_ext.ap())
        nc.sync.dma_start(wklrep[:], wkl_ext.ap())
        nc.sync.dma_start(wrr[:], wrr_ext.ap())
        nc.sync.dma_start(wrr8[:], wrr8_ext.ap())
        nc.sync.dma_start(bq_sb[:], bq_ext.ap())
        nc.sync.dma_start(bk_sb[:], bk_ext.ap())
        nc.sync.dma_start(bv_sb[:], bv_ext.ap())
        nc.sync.dma_start(br_sb[:], br_ext.ap())
        nc.sync.dma_start(wq_sb[:, 0, :], wq_ext.ap()[0:P, :])
        # quarter 0 is split across both queues so the first GEMMs unblock
        # as early as possible; wq t-blocks 1..7 follow on sync.
        for k in range(0, T, 2):
            nc.sync.dma_start(
                xt_sb[:, k, 0:NP], xt_ext.ap()[k * P : (k + 1) * P, 0:NP]
            )
        for k in range(1, T, 2):
            nc.gpsimd.dma_start(
                xt_sb[:, k, 0:NP], xt_ext.ap()[k * P : (k + 1) * P, 0:NP]
            )
        for t in range(1, T):
            nc.sync.dma_start(wq_sb[:, t, :], wq_ext.ap()[t * P : (t + 1) * P, :])
        # gpsimd queue: remaining xt quarters (landing in consumption
        # order), then the K/V-phase weights. xt8 is derived on-device.
        for g in range(1, NG):
            sl = slice(g * NP, (g + 1) * NP)
            for k in range(T):
                nc.gpsimd.dma_start(
                    xt_sb[:, k, sl], xt_ext.ap()[k * P : (k + 1) * P, sl]
                )
        for k in range(T):
            rsl = slice(k * P, (k + 1) * P)
            nc.gpsimd.dma_start(wk_sb[:, k, :], wk_ext.ap()[rsl, :])
        for k in range(T):
            rsl = slice(k * P, (k + 1) * P)
            nc.gpsimd.dma_start(wvt_sb[:, k, :], wvt_ext.ap()[rsl, :])

        # ---- warm-up during the DMA window: HAM un-throttle + exp table ----
        nc.vector.memset(scratch[:], 0.0)
        warm_ps = pslg.tile([P, NP], F32, name="plg", tag="plg")
        for _ in range(18):
            nc.tensor.matmul(warm_ps[:, 0:NC_], scratch[:, 0:P], scratch[:],
                             start=True, stop=True)
        nc.scalar.activation(scr_e[:], warm_ps[:, 0:NC_], AF.Exp,
                             bias=0.0, scale=1.0)
        nc.vector.tensor_copy(bk16_sb[:], bk_sb[:])
        for _ in range(45):
            nc.tensor.matmul(
                warm_ps[:, 0:NC_], wq_sb[:, 0, 0:P],
                wq_sb[:, 0, 0:NC_], start=True, stop=True,
            )

        # ================= Q phase: s-quarter outer =================
        pend = None  # deferred logit matmul for the previous pair

        def emit_logit_q(t, g, qtc):
            plg = pslg.tile([P, NP], F32, name="plg", tag="plg")
            nc.tensor.matmul(plg[:, 0:NC_], wqlrep[:], qtc[:, 0:NC_],
                             start=True, stop=True)
            nc.tensor.matmul(plg[:, NC_:NP], wqlrep[:], qtc[:, NC_:NP],
                             start=True, stop=True)
            ec = e_pool.tile([P, NP], BF16, name="ec", tag="ec")
            nc.scalar.activation(
                ec[:], plg[:], AF.Exp, bias=0.0, scale=1.0,
                accum_out=zq_sb[:, t, g : g + 1],
            )
            nc.vector.scalar_tensor_tensor(
                ec[:], ec[:], 1.0, qtc[:], ALU.mult, ALU.mult,
                accum_out=gqp_sb[:, t, g : g + 1],
            )

        def finalize_q(t):
            ztot = small_pool.tile([P, 1], F32, name="ztot", tag="ztot")
            nc.vector.reduce_sum(ztot, zq_sb[:, t, :], axis=AX)
            recip = small_pool.tile([P, 1], F32, name="recip", tag="recip")
            nc.vector.reciprocal(recip, ztot)
            graw = small_pool.tile([P, 1], F32, name="graw", tag="graw")
            nc.vector.reduce_sum(graw, gqp_sb[:, t, :], axis=AX)
            nc.vector.tensor_mul(gq_all[:, t : t + 1], graw, recip)
            # eff_kl for the K phase, built as soon as gq[t] exists
            nc.vector.tensor_scalar_mul(
                effk_sb[:, t, :], wklrep[:], gq_all[:, t : t + 1]
            )

        for g in range(NG):
            sl = slice(g * NP, (g + 1) * NP)
            for t in range(T):
                pch = psum.tile([P, NP], F32, name="pch", tag="pch")
                for h in range(2):
                    hsl = slice(g * NP + h * NC_, g * NP + (h + 1) * NC_)
                    psl = slice(h * NC_, (h + 1) * NC_)
                    for k in range(T):
                        nc.tensor.matmul(
                            pch[:, psl],
                            wq_sb[:, t, k * P : (k + 1) * P],
                            xt_sb[:, k, hsl],
                            start=(k == 0),
                            stop=(k == T - 1),
                        )
                qtc = ch_pool.tile([P, NP], BF16, name="qtc", tag="qtc")
                nc.scalar.activation(
                    qtc[:], pch[:], AF.Identity, bias=bq_sb[:, t : t + 1],
                    scale=1.0,
                )
                nc.sync.dma_start(out_ext.ap()[t * P : (t + 1) * P, sl], qtc[:])
                if pend is not None:
                    emit_logit_q(*pend)
                pend = (t, g, qtc)
                # derive the fp8 copy of xt on spare DVE capacity. One
                # k-tile per iteration through quarter 2; quarter 3's
                # conversions are front-loaded so the phase tail keeps DVE
                # free for the finalizers and the K-phase rampup.
                if g < 3:
                    c = g * T + t
                    nc.vector.tensor_copy(
                        xt8_sb[:, c % T, (c // T) * NP : (c // T + 1) * NP],
                        xt_sb[:, c % T, (c // T) * NP : (c // T + 1) * NP],
                    )
                elif t < 4:
                    for c in (24 + 2 * t, 25 + 2 * t):
                        nc.vector.tensor_copy(
                            xt8_sb[:, c % T, (c // T) * NP : (c // T + 1) * NP],
                            xt_sb[:, c % T, (c // T) * NP : (c // T + 1) * NP],
                        )
                if g == NG - 1 and t > 0:
                    finalize_q(t - 1)
        emit_logit_q(*pend)
        finalize_q(T - 1)

        # ================= K phase: tile outer =================
        def emit_logit_k(t, g, pc):
            # pc holds the RAW 64*k chunk; the bk part of the logit is the
            # per-column constant cb[t] (folded into the exp bias), and the
            # bk part of the pooling numerator folds into the stt op0.
            plg = pslg.tile([P, NP], F32, name="plg", tag="plg")
            nc.tensor.matmul(plg[:, 0:NC_], effk_sb[:, t, :], pc[:, 0:NC_],
                             start=True, stop=True)
            nc.tensor.matmul(plg[:, NC_:NP], effk_sb[:, t, :], pc[:, NC_:NP],
                             start=True, stop=True)
            ec = e_pool.tile([P, NP], BF16, name="ec", tag="ec")
            nc.scalar.activation(
                ec[:], plg[:], AF.Exp, bias=cb_sb[:, t : t + 1], scale=1.0,
                accum_out=zk_sb[:, t, g : g + 1],
            )
            nc.vector.scalar_tensor_tensor(
                ec[:], pc[:], bk_sb[:, t : t + 1], ec[:], ALU.add, ALU.mult,
                accum_out=gkp_sb[:, t, g : g + 1],
            )

        def finalize_k(t):
            """gk_all[:, t] = gq * (sum_g gkp / (64 * sum_g zk)) — true gk."""
            ztot = small_pool.tile([P, 1], F32, name="ztot", tag="ztot")
            nc.vector.reduce_sum(ztot, zk_sb[:, t, :], axis=AX)
            nc.vector.tensor_scalar_mul(ztot, ztot, W8SCALE)
            recip = small_pool.tile([P, 1], F32, name="recip", tag="recip")
            nc.vector.reciprocal(recip, ztot)
            graw = small_pool.tile([P, 1], F32, name="graw", tag="graw")
            nc.vector.reduce_sum(graw, gkp_sb[:, t, :], axis=AX)
            nc.vector.tensor_mul(graw, graw, recip)
            nc.vector.tensor_mul(gk_all[:, t : t + 1], graw, gq_all[:, t : t + 1])

        def build_wcomb(j):
            """wcomb8[:, :, j-cols] = 2^18 * Wv @ diag(gk_j) @ Wr, block j."""
            nc.vector.tensor_scalar_mul(gk8c[:], gk_all[:, j : j + 1], GK8)
            gkwr = eff_pool.tile([P, P], FP8, name="gkwr", tag="gkwr")
            nc.vector.tensor_scalar_mul(gkwr[:], wrr8[:], gk8c[:, 0:1])
            pw = pslg.tile([P, NP], F32, name="plg", tag="plg")
            for tt in range(T):
                nc.tensor.matmul(
                    pw[:, tt * P : (tt + 1) * P],
                    wvt_sb[:, j, tt * P : (tt + 1) * P], gkwr[:],
                    start=True, stop=True,
                )
            nc.scalar.activation(
                wcomb_sb[:, :, j * P : (j + 1) * P], pw[:],
                AF.Identity, bias=0.0, scale=WCE,
            )
            # bias2_j = Wr^T (bv*gk)_j + br    (true scale)
            nc.vector.tensor_mul(
                bvg_sb[:, j : j + 1], bv_sb[:, j : j + 1], gk_all[:, j : j + 1]
            )
            pb = pslg.tile([P, NP], F32, name="plg", tag="plg")
            nc.tensor.matmul(
                pb[:, 0:1], wrr[:], bvg_sb[:, j : j + 1], start=True, stop=True
            )
            nc.scalar.activation(
                bias2_sb[:, j : j + 1], pb[:, 0:1], AF.Identity,
                bias=br_sb[:, 0:1], scale=1.0,
            )

        pendq = []
        for t in range(T):
            # cb[t] = effk[t]^T bk64 — the constant column of the beta logit
            cbp = pslg.tile([P, NP], F32, name="plg", tag="plg")
            nc.tensor.matmul(cbp[:, 0:1], effk_sb[:, t, :],
                             bk16_sb[:, t : t + 1], start=True, stop=True)
            nc.scalar.activation(cb_sb[:, t : t + 1], cbp[:, 0:1],
                                 AF.Identity, bias=0.0, scale=1.0)
            for g in range(NG):
                pch = psum.tile([P, NP], F32, name="pch", tag="pch")
                for h in range(2):
                    hsl = slice(g * NP + h * NC_, g * NP + (h + 1) * NC_)
                    psl = slice(h * NC_, (h + 1) * NC_)
                    for k in range(0, T, 2):
                        nc.tensor.matmul(
                            pch[:, psl],
                            wk_sb[:, k : k + 2, t * P : (t + 1) * P],
                            xt8_sb[:, k : k + 2, hsl],
                            start=(k == 0),
                            stop=(k == T - 2),
                            perf_mode=mybir.MatmulPerfMode.DoubleRow,
                        )
                pc = ch_pool.tile([P, NP], BF16, name="qtc", tag="qtc")
                if g % 2 == 0:
                    nc.scalar.activation(pc[:], pch[:], AF.Identity,
                                         bias=0.0, scale=1.0)
                else:
                    nc.vector.tensor_copy(pc[:], pch[:])
                pendq.append((t, g, pc))
                if len(pendq) > 2:
                    emit_logit_k(*pendq.pop(0))
                if t > 0:
                    if g == 2:
                        finalize_k(t - 1)
                    elif g == 3:
                        build_wcomb(t - 1)

        # ================= V phase: pure fp8 GEMM with folded gate =======
        first_pair_done = False
        for t in range(T):
            for g in range(NG):
                sl = slice(g * NP, (g + 1) * NP)
                pch = psum.tile([P, NP], F32, name="pch", tag="pch")
                for h in range(2):
                    hsl = slice(g * NP + h * NC_, g * NP + (h + 1) * NC_)
                    psl = slice(h * NC_, (h + 1) * NC_)
                    for k in range(0, T, 2):
                        nc.tensor.matmul(
                            pch[:, psl],
                            wcomb_sb[:, k : k + 2, t * P : (t + 1) * P],
                            xt8_sb[:, k : k + 2, hsl],
                            start=(k == 0),
                            stop=(k == T - 2),
                            perf_mode=mybir.MatmulPerfMode.DoubleRow,
                        )
                if not first_pair_done:
                    # last block's logits + finalize ride behind V tile 0's
                    # first GEMM pair so the PE never idles at the boundary
                    while pendq:
                        emit_logit_k(*pendq.pop(0))
                    finalize_k(T - 1)
                    build_wcomb(T - 1)
                    first_pair_done = True
                stg = ch_pool.tile([P, NP], BF16, name="qtc", tag="qtc")
                nc.scalar.activation(
                    stg[:], pch[:], AF.Identity,
                    bias=bias2_sb[:, t : t + 1], scale=VDE,
                )
                nc.sync.dma_start(out2_ext.ap()[t * P : (t + 1) * P, sl], stg[:])

    nc.compile()
    return nc


def _prep_shared(inputs):
    """Host-side prep of the replicated (weight) arrays."""
    sc = 0.125  # 1/sqrt(HD)

    def rep_logit(w, scale):
        m = np.zeros((P, P), dtype=np.float32)
        ws = w.astype(np.float32) * scale
        m[:HD, :HD] = ws[:, None]
        m[HD:, HD:] = ws[:, None]
        return m.astype(BF)

    def bias_pp(b):
        return np.ascontiguousarray(b.astype(np.float32).reshape(T, P).T)

    wrrf = np.zeros((P, P), dtype=np.float32)
    wr = inputs["Wr"].astype(np.float32)
    wrrf[:HD, :HD] = wr
    wrrf[HD:, HD:] = wr

    wq_tmaj = (
        inputs["Wq"].astype(np.float32)
        .reshape(T, P, T, P).transpose(2, 1, 0, 3).reshape(D, D)
    )
    return {
        "wq": np.ascontiguousarray(wq_tmaj.astype(BF)),
        "wk": np.ascontiguousarray(
            (inputs["Wk"].astype(np.float32) * W8SCALE).astype(F8)
        ),
        "wvt": np.ascontiguousarray(
            (inputs["Wv"].astype(np.float32).T * W8SCALE).astype(F8)
        ),
        "bq": bias_pp(inputs["bq"]),
        "bk": bias_pp(inputs["bk"]) * np.float32(W8SCALE),
        "bv": bias_pp(inputs["bv"]),
        "wqlrep": rep_logit(inputs["wql"], sc),
        "wklrep": rep_logit(inputs["wkl"], sc / W8SCALE),
        "wrr": wrrf.astype(BF),
        "wrr8": (wrrf * W8SCALE).astype(F8),
        "br": np.ascontiguousarray(
            np.tile(inputs["br"].astype(np.float32), 2).reshape(P, 1)
        ),
    }


def _get_nc():
    if "nc" not in _CACHE:
        _CACHE["nc"] = _build()
    return _CACHE["nc"]


def _run(inputs, trace=False):
    nc = _get_nc()
    shared = _prep_shared(inputs)
    X = inputs["X"]
    in_maps = []
    for b in range(N_CORES):
        m = dict(shared)
        m["xt"] = np.ascontiguousarray(X[b].T).astype(BF)
        in_maps.append(m)
    if trace:
        _install_profile_hook()
    res = run_bass_kernel_spmd(nc, in_maps, list(range(N_CORES)), trace=trace)
    out = np.empty((B, S, D), dtype=np.float32)
    for b in range(N_CORES):
        r = res.results[b]
        out[b] = (
            np.asarray(r["out"]).astype(np.float32)
            + np.asarray(r["out2"]).astype(np.float32)
        ).T
    return out, res


def _install_profile_hook():
    import antenv

    if "antenv.axon_hooks" not in sys.modules:
        mod = types.ModuleType("antenv.axon_hooks")
        mod._hook = None
        mod.set_axon_ntff_profile_hook = lambda h: setattr(mod, "_hook", h)
        mod.get_axon_ntff_profile_hook = lambda: mod._hook
        sys.modules["antenv.axon_hooks"] = mod
        antenv.axon_hooks = mod
    hooks = sys.modules["antenv.axon_hooks"]
    if hooks.get_axon_ntff_profile_hook() is None:
        from trn_agent_boot.trn_boot import _ntff_profile_via_ctypes

        hooks.set_axon_ntff_profile_hook(
            _ntff_profile_via_ctypes("/opt/axon/libaxon_pjrt.so")
        )
    import concourse.bass_utils as bass_utils

    bass_utils.upload_artifacts = lambda tmpdir: f"local:{tmpdir}"


def kernel(**inputs) -> np.ndarray:
    out, _ = _run(inputs, trace=False)
    return out


# revision 23
# speedup vs baseline: 1.0494x; 1.0494x over previous
"""Trainium2 Bass kernel for nn_AdditiveAttention (B=8, S=4096, D=1024, H=16).

Sharding: pure data-parallel over batch — 8 NeuronCores, one batch element
per core, weights replicated. No collectives.

Per-core layout: everything transposed (d on partitions, s on free).

v3 structure:
  - Q phase is s-quarter-outer: each 2.1 MB quarter-column of xt is
    consumed as its DMAs land, so the PE starts ~12us into the kernel and
    streams from there. Warm-up matmuls bridge the DMA window (HAM
    un-throttle + ACT exp-table load).
  - All per-chunk ops are 1024-wide (two 512 psum banks per tile) to
    amortize the ~300ns fixed per-op engine overhead.
  - Pooling is incremental: ACT Exp writes accum_out Z-partials, a fused
    DVE scalar_tensor_tensor(mult, mult, accum_out) does the numerator.
    K-phase psum evacuations ride DVE (tensor_scalar_add) to keep ACT
    at the exp-only load.
  - The elementwise V gate u = (v+bv)*gk and the r-projection fold into
    the weights on-device: Wcomb = Wv @ diag(gk_h) @ Wr per head (64
    N=128 fp8 matmuls from Wv^T, built per head-pair block as that
    block's gk finalizes inside the K phase), so the V phase is a single
    streaming fp8 DoubleRow GEMM producing the rt output directly.
  - Outputs are bf16 (q-residual and rt part summed on host in f32).
"""

import sys
import types

import numpy as np
import ml_dtypes

from contextlib import ExitStack

import concourse.bass as bass
import concourse.tile as tile
from concourse import bacc, mybir
from concourse.bass_utils import run_bass_kernel_spmd

B, S, D, H, HD = 8, 4096, 1024, 16, 64
P = 128          # partitions
T = D // P       # 8 d-tiles
NC_ = 512        # psum bank free size
NP = 1024        # paired op width
NG = S // NP     # 4 quarter/pair groups
N_CORES = 8
BF16 = mybir.dt.bfloat16
FP8 = mybir.dt.float8e4
F32 = mybir.dt.float32
W8SCALE = 64.0   # host scales Wk/WvT/Wr by this into e4m3 normal range
GK8 = 256.0      # device scale for gk -> fp8 operand
WCE = 2.0 ** -4  # psWC -> wcomb8 evacuation scale
VDE = 2.0 ** -18 # V-phase psum -> output scale (undoes 64*64*256*WCE)
BF = ml_dtypes.bfloat16
F8 = ml_dtypes.float8_e4m3

_CACHE = {}


def _build():
    nc = bacc.Bacc(
        "TRN2", target_bir_lowering=False, debug=False, num_devices=N_CORES
    )
    xt_ext = nc.declare_dram_parameter("xt", [D, S], BF16, isOutput=False)
    wq_ext = nc.declare_dram_parameter("wq", [D, D], BF16, isOutput=False)
    wk_ext = nc.declare_dram_parameter("wk", [D, D], FP8, isOutput=False)
    wvt_ext = nc.declare_dram_parameter("wvt", [D, D], FP8, isOutput=False)
    bq_ext = nc.declare_dram_parameter("bq", [P, T], F32, isOutput=False)
    bk_ext = nc.declare_dram_parameter("bk", [P, T], F32, isOutput=False)
    bv_ext = nc.declare_dram_parameter("bv", [P, T], F32, isOutput=False)
    wql_ext = nc.declare_dram_parameter("wqlrep", [P, P], BF16, isOutput=False)
    wkl_ext = nc.declare_dram_parameter("wklrep", [P, P], BF16, isOutput=False)
    wrr_ext = nc.declare_dram_parameter("wrr", [P, P], BF16, isOutput=False)
    wrr8_ext = nc.declare_dram_parameter("wrr8", [P, P], FP8, isOutput=False)
    br_ext = nc.declare_dram_parameter("br", [P, 1], F32, isOutput=False)
    out_ext = nc.declare_dram_parameter("out", [D, S], BF16, isOutput=True)
    out2_ext = nc.declare_dram_parameter("out2", [D, S], BF16, isOutput=True)

    AX = mybir.AxisListType.X
    ALU = mybir.AluOpType
    AF = mybir.ActivationFunctionType

    with tile.TileContext(nc) as tc, ExitStack() as ctx:
        singles = ctx.enter_context(tc.tile_pool(name="singles", bufs=1))
        psum = ctx.enter_context(tc.tile_pool(name="psum", bufs=2, space="PSUM"))
        pslg = ctx.enter_context(tc.tile_pool(name="pslg", bufs=2, space="PSUM"))
        ch_pool = ctx.enter_context(tc.tile_pool(name="chpool", bufs=5))
        e_pool = ctx.enter_context(tc.tile_pool(name="epool", bufs=5))
        eff_pool = ctx.enter_context(tc.tile_pool(name="eff", bufs=2))
        small_pool = ctx.enter_context(tc.tile_pool(name="small", bufs=4))

        # ---- resident tiles ----
        xt_sb = singles.tile([P, T, S], BF16, name="xt", tag="xt")
        xt8_sb = singles.tile([P, T, S], FP8, name="xt8", tag="xt8")
        wq_sb = singles.tile([P, T, D], BF16, name="wq", tag="wq")
        wk_sb = singles.tile([P, T, D], FP8, name="wk", tag="wk")
        wvt_sb = singles.tile([P, T, D], FP8, name="wvt", tag="wvt")
        wcomb_sb = singles.tile([P, T, D], FP8, name="wcomb", tag="wcomb")
        wqlrep = singles.tile([P, P], BF16, name="wqlrep", tag="wqlrep")
        wklrep = singles.tile([P, P], BF16, name="wklrep", tag="wklrep")
        wrr = singles.tile([P, P], BF16, name="wrr", tag="wrr")
        wrr8 = singles.tile([P, P], FP8, name="wrr8", tag="wrr8")
        bq_sb = singles.tile([P, T], F32, name="bq", tag="bq")
        bk_sb = singles.tile([P, T], F32, name="bk", tag="bk")
        bk16_sb = singles.tile([P, T], BF16, name="bk16", tag="bk16")
        cb_sb = singles.tile([P, T], F32, name="cb", tag="cb")
        bv_sb = singles.tile([P, T], F32, name="bv", tag="bv")
        bvg_sb = singles.tile([P, T], BF16, name="bvg", tag="bvg")
        br_sb = singles.tile([P, 1], F32, name="br", tag="br")
        bias2_sb = singles.tile([P, T], F32, name="bias2", tag="bias2")
        zq_sb = singles.tile([P, T, NG], F32, name="zq", tag="zq")
        gqp_sb = singles.tile([P, T, NG], F32, name="gqp", tag="gqp")
        zk_sb = singles.tile([P, T, NG], F32, name="zk", tag="zk")
        gkp_sb = singles.tile([P, T, NG], F32, name="gkp", tag="gkp")
        gq_all = singles.tile([P, T], F32, name="gq", tag="gq")
        gk_all = singles.tile([P, T], F32, name="gk", tag="gk")
        gk8c = singles.tile([P, 1], F32, name="gk8c", tag="gk8c")
        effk_sb = singles.tile([P, T, P], BF16, name="effk", tag="effk")
        scratch = singles.tile([P, NC_], BF16, name="scr", tag="scr")
        scr_e = singles.tile([P, NC_], BF16, name="scre", tag="scre")

        # ---- DMA issue ----
        # sync queue: small weights, then wq in t-major blocks (each block
        # unlocks one output tile's GEMMs — block 0 lands ~5us).
        nc.sync.dma_start(wqlrep[:], wql_ext.ap())
        nc.sync.dma_start(wklrep[:], wkl_ext.ap())
        nc.sync.dma_start(wrr[:], wrr_ext.ap())
        nc.sync.dma_start(wrr8[:], wrr8_ext.ap())
        nc.sync.dma_start(bq_sb[:], bq_ext.ap())
        nc.sync.dma_start(bk_sb[:], bk_ext.ap())
        nc.sync.dma_start(bv_sb[:], bv_ext.ap())
        nc.sync.dma_start(br_sb[:], br_ext.ap())
        nc.sync.dma_start(wq_sb[:, 0, :], wq_ext.ap()[0:P, :])
        # quarter 0 is split across both queues so the first GEMMs unblock
        # as early as possible; wq t-blocks 1..7 follow on sync.
        for k in range(0, T, 2):
            nc.sync.dma_start(
                xt_sb[:, k, 0:NP], xt_ext.ap()[k * P : (k + 1) * P, 0:NP]
            )
        for k in range(1, T, 2):
            nc.gpsimd.dma_start(
                xt_sb[:, k, 0:NP], xt_ext.ap()[k * P : (k + 1) * P, 0:NP]
            )
        for t in range(1, T):
            nc.sync.dma_start(wq_sb[:, t, :], wq_ext.ap()[t * P : (t + 1) * P, :])
        # gpsimd queue: remaining xt quarters (landing in consumption
        # order), then the K/V-phase weights. xt8 is derived on-device.
        for g in range(1, NG):
            sl = slice(g * NP, (g + 1) * NP)
            for k in range(T):
                nc.gpsimd.dma_start(
                    xt_sb[:, k, sl], xt_ext.ap()[k * P : (k + 1) * P, sl]
                )
        for k in range(T):
            rsl = slice(k * P, (k + 1) * P)
            nc.gpsimd.dma_start(wk_sb[:, k, :], wk_ext.ap()[rsl, :])
        for k in range(T):
            rsl = slice(k * P, (k + 1) * P)
            nc.gpsimd.dma_start(wvt_sb[:, k, :], wvt_ext.ap()[rsl, :])

        # ---- warm-up during the DMA window: HAM un-throttle + exp table ----
        nc.vector.memset(scratch[:], 0.0)
        warm_ps = pslg.tile([P, NP], F32, name="plg", tag="plg")
        for _ in range(18):
            nc.tensor.matmul(warm_ps[:, 0:NC_], scratch[:, 0:P], scratch[:],
                             start=True, stop=True)
        nc.scalar.activation(scr_e[:], warm_ps[:, 0:NC_], AF.Exp,
                             bias=0.0, scale=1.0)
        nc.vector.tensor_copy(bk16_sb[:], bk_sb[:])
        for _ in range(20):
            nc.tensor.matmul(
                warm_ps[:, 0:NC_], wq_sb[:, 0, 0:P],
                wq_sb[:, 0, 0:NC_], start=True, stop=True,
            )

        # ================= Q phase: s-quarter outer =================
        pend = None  # deferred logit matmul for the previous pair

        def emit_logit_q(t, g, qtc):
            plg = pslg.tile([P, NP], F32, name="plg", tag="plg")
            nc.tensor.matmul(plg[:, 0:NC_], wqlrep[:], qtc[:, 0:NC_],
                             start=True, stop=True)
            nc.tensor.matmul(plg[:, NC_:NP], wqlrep[:], qtc[:, NC_:NP],
                             start=True, stop=True)
            ec = e_pool.tile([P, NP], BF16, name="ec", tag="ec")
            nc.scalar.activation(
                ec[:], plg[:], AF.Exp, bias=0.0, scale=1.0,
                accum_out=zq_sb[:, t, g : g + 1],
            )
            nc.vector.scalar_tensor_tensor(
                ec[:], ec[:], 1.0, qtc[:], ALU.mult, ALU.mult,
                accum_out=gqp_sb[:, t, g : g + 1],
            )

        def finalize_q(t):
            ztot = small_pool.tile([P, 1], F32, name="ztot", tag="ztot")
            nc.vector.reduce_sum(ztot, zq_sb[:, t, :], axis=AX)
            recip = small_pool.tile([P, 1], F32, name="recip", tag="recip")
            nc.vector.reciprocal(recip, ztot)
            graw = small_pool.tile([P, 1], F32, name="graw", tag="graw")
            nc.vector.reduce_sum(graw, gqp_sb[:, t, :], axis=AX)
            nc.vector.tensor_mul(gq_all[:, t : t + 1], graw, recip)
            # eff_kl for the K phase, built as soon as gq[t] exists
            nc.vector.tensor_scalar_mul(
                effk_sb[:, t, :], wklrep[:], gq_all[:, t : t + 1]
            )

        for g in range(NG):
            sl = slice(g * NP, (g + 1) * NP)
            for t in range(T):
                pch = psum.tile([P, NP], F32, name="pch", tag="pch")
                for h in range(2):
                    hsl = slice(g * NP + h * NC_, g * NP + (h + 1) * NC_)
                    psl = slice(h * NC_, (h + 1) * NC_)
                    for k in range(T):
                        nc.tensor.matmul(
                            pch[:, psl],
                            wq_sb[:, t, k * P : (k + 1) * P],
                            xt_sb[:, k, hsl],
                            start=(k == 0),
                            stop=(k == T - 1),
                        )
                qtc = ch_pool.tile([P, NP], BF16, name="qtc", tag="qtc")
                nc.scalar.activation(
                    qtc[:], pch[:], AF.Identity, bias=bq_sb[:, t : t + 1],
                    scale=1.0,
                )
                nc.sync.dma_start(out_ext.ap()[t * P : (t + 1) * P, sl], qtc[:])
                if pend is not None:
                    emit_logit_q(*pend)
                pend = (t, g, qtc)
                # derive the fp8 copy of xt on spare DVE capacity. One
                # k-tile per iteration through quarter 2; quarter 3's
                # conversions are front-loaded so the phase tail keeps DVE
                # free for the finalizers and the K-phase rampup.
                if g < 3:
                    c = g * T + t
                    nc.vector.tensor_copy(
                        xt8_sb[:, c % T, (c // T) * NP : (c // T + 1) * NP],
                        xt_sb[:, c % T, (c // T) * NP : (c // T + 1) * NP],
                    )
                elif t < 4:
                    for c in (24 + 2 * t, 25 + 2 * t):
                        nc.vector.tensor_copy(
                            xt8_sb[:, c % T, (c // T) * NP : (c // T + 1) * NP],
                            xt_sb[:, c % T, (c // T) * NP : (c // T + 1) * NP],
                        )
                if g == NG - 1 and t > 0:
                    finalize_q(t - 1)
        emit_logit_q(*pend)
        finalize_q(T - 1)

        # ================= K phase: tile outer =================
        def emit_logit_k(t, g, pc):
            # pc holds the RAW 64*k chunk; the bk part of the logit is the
            # per-column constant cb[t] (folded into the exp bias), and the
            # bk part of the pooling numerator folds into the stt op0.
            plg = pslg.tile([P, NP], F32, name="plg", tag="plg")
            nc.tensor.matmul(plg[:, 0:NC_], effk_sb[:, t, :], pc[:, 0:NC_],
                             start=True, stop=True)
            nc.tensor.matmul(plg[:, NC_:NP], effk_sb[:, t, :], pc[:, NC_:NP],
                             start=True, stop=True)
            ec = e_pool.tile([P, NP], BF16, name="ec", tag="ec")
            nc.scalar.activation(
                ec[:], plg[:], AF.Exp, bias=cb_sb[:, t : t + 1], scale=1.0,
                accum_out=zk_sb[:, t, g : g + 1],
            )
            nc.vector.scalar_tensor_tensor(
                ec[:], pc[:], bk_sb[:, t : t + 1], ec[:], ALU.add, ALU.mult,
                accum_out=gkp_sb[:, t, g : g + 1],
            )

        def finalize_k(t):
            """gk_all[:, t] = gq * (sum_g gkp / (64 * sum_g zk)) — true gk."""
            ztot = small_pool.tile([P, 1], F32, name="ztot", tag="ztot")
            nc.vector.reduce_sum(ztot, zk_sb[:, t, :], axis=AX)
            nc.vector.tensor_scalar_mul(ztot, ztot, W8SCALE)
            recip = small_pool.tile([P, 1], F32, name="recip", tag="recip")
            nc.vector.reciprocal(recip, ztot)
            graw = small_pool.tile([P, 1], F32, name="graw", tag="graw")
            nc.vector.reduce_sum(graw, gkp_sb[:, t, :], axis=AX)
            nc.vector.tensor_mul(graw, graw, recip)
            nc.vector.tensor_mul(gk_all[:, t : t + 1], graw, gq_all[:, t : t + 1])

        def build_wcomb(j):
            """wcomb8[:, :, j-cols] = 2^18 * Wv @ diag(gk_j) @ Wr, block j."""
            nc.vector.tensor_scalar_mul(gk8c[:], gk_all[:, j : j + 1], GK8)
            gkwr = eff_pool.tile([P, P], FP8, name="gkwr", tag="gkwr")
            nc.vector.tensor_scalar_mul(gkwr[:], wrr8[:], gk8c[:, 0:1])
            pw = pslg.tile([P, NP], F32, name="plg", tag="plg")
            for tt in range(T):
                nc.tensor.matmul(
                    pw[:, tt * P : (tt + 1) * P],
                    wvt_sb[:, j, tt * P : (tt + 1) * P], gkwr[:],
                    start=True, stop=True,
                )
            nc.scalar.activation(
                wcomb_sb[:, :, j * P : (j + 1) * P], pw[:],
                AF.Identity, bias=0.0, scale=WCE,
            )
            # bias2_j = Wr^T (bv*gk)_j + br    (true scale)
            nc.vector.tensor_mul(
                bvg_sb[:, j : j + 1], bv_sb[:, j : j + 1], gk_all[:, j : j + 1]
            )
            pb = pslg.tile([P, NP], F32, name="plg", tag="plg")
            nc.tensor.matmul(
                pb[:, 0:1], wrr[:], bvg_sb[:, j : j + 1], start=True, stop=True
            )
            nc.scalar.activation(
                bias2_sb[:, j : j + 1], pb[:, 0:1], AF.Identity,
                bias=br_sb[:, 0:1], scale=1.0,
            )

        pendq = []
        for t in range(T):
            # cb[t] = effk[t]^T bk64 — the constant column of the beta logit
            cbp = pslg.tile([P, NP], F32, name="plg", tag="plg")
            nc.tensor.matmul(cbp[:, 0:1], effk_sb[:, t, :],
                             bk16_sb[:, t : t + 1], start=True, stop=True)
            nc.scalar.activation(cb_sb[:, t : t + 1], cbp[:, 0:1],
                                 AF.Identity, bias=0.0, scale=1.0)
            for g in range(NG):
                pch = psum.tile([P, NP], F32, name="pch", tag="pch")
                for h in range(2):
                    hsl = slice(g * NP + h * NC_, g * NP + (h + 1) * NC_)
                    psl = slice(h * NC_, (h + 1) * NC_)
                    for k in range(0, T, 2):
                        nc.tensor.matmul(
                            pch[:, psl],
                            wk_sb[:, k : k + 2, t * P : (t + 1) * P],
                            xt8_sb[:, k : k + 2, hsl],
                            start=(k == 0),
                            stop=(k == T - 2),
                            perf_mode=mybir.MatmulPerfMode.DoubleRow,
                        )
                pc = ch_pool.tile([P, NP], BF16, name="qtc", tag="qtc")
                if g % 2 == 0:
                    nc.scalar.activation(pc[:], pch[:], AF.Identity,
                                         bias=0.0, scale=1.0)
                else:
                    nc.vector.tensor_copy(pc[:], pch[:])
                pendq.append((t, g, pc))
                if len(pendq) > 2:
                    emit_logit_k(*pendq.pop(0))
                if t > 0:
                    if g == 2:
                        finalize_k(t - 1)
                    elif g == 3:
                        build_wcomb(t - 1)

        # ================= V phase: pure fp8 GEMM with folded gate =======
        first_pair_done = False
        for t in range(T):
            for g in range(NG):
                sl = slice(g * NP, (g + 1) * NP)
                pch = psum.tile([P, NP], F32, name="pch", tag="pch")
                for h in range(2):
                    hsl = slice(g * NP + h * NC_, g * NP + (h + 1) * NC_)
                    psl = slice(h * NC_, (h + 1) * NC_)
                    for k in range(0, T, 2):
                        nc.tensor.matmul(
                            pch[:, psl],
                            wcomb_sb[:, k : k + 2, t * P : (t + 1) * P],
                            xt8_sb[:, k : k + 2, hsl],
                            start=(k == 0),
                            stop=(k == T - 2),
                            perf_mode=mybir.MatmulPerfMode.DoubleRow,
                        )
                if not first_pair_done:
                    # last block's logits + finalize ride behind V tile 0's
                    # first GEMM pair so the PE never idles at the boundary
                    while pendq:
                        emit_logit_k(*pendq.pop(0))
                    finalize_k(T - 1)
                    build_wcomb(T - 1)
                    first_pair_done = True
                stg = ch_pool.tile([P, NP], BF16, name="qtc", tag="qtc")
                nc.scalar.activation(
                    stg[:], pch[:], AF.Identity,
                    bias=bias2_sb[:, t : t + 1], scale=VDE,
                )
                nc.sync.dma_start(out2_ext.ap()[t * P : (t + 1) * P, sl], stg[:])

    nc.compile()
    return nc


def _prep_shared(inputs):
    """Host-side prep of the replicated (weight) arrays."""
    sc = 0.125  # 1/sqrt(HD)

    def rep_logit(w, scale):
        m = np.zeros((P, P), dtype=np.float32)
        ws = w.astype(np.float32) * scale
        m[:HD, :HD] = ws[:, None]
        m[HD:, HD:] = ws[:, None]
        return m.astype(BF)

    def bias_pp(b):
        return np.ascontiguousarray(b.astype(np.float32).reshape(T, P).T)

    wrrf = np.zeros((P, P), dtype=np.float32)
    wr = inputs["Wr"].astype(np.float32)
    wrrf[:HD, :HD] = wr
    wrrf[HD:, HD:] = wr

    wq_tmaj = (
        inputs["Wq"].astype(np.float32)
        .reshape(T, P, T, P).transpose(2, 1, 0, 3).reshape(D, D)
    )
    return {
        "wq": np.ascontiguousarray(wq_tmaj.astype(BF)),
        "wk": np.ascontiguousarray(
            (inputs["Wk"].astype(np.float32) * W8SCALE).astype(F8)
        ),
        "wvt": np.ascontiguousarray(
            (inputs["Wv"].astype(np.float32).T * W8SCALE).astype(F8)
        ),
        "bq": bias_pp(inputs["bq"]),
        "bk": bias_pp(inputs["bk"]) * np.float32(W8SCALE),
        "bv": bias_pp(inputs["bv"]),
        "wqlrep": rep_logit(inputs["wql"], sc),
        "wklrep": rep_logit(inputs["wkl"], sc / W8SCALE),
        "wrr": wrrf.astype(BF),
        "wrr8": (wrrf * W8SCALE).astype(F8),
        "br": np.ascontiguousarray(
            np.tile(inputs["br"].astype(np.float32), 2).reshape(P, 1)
        ),
    }


def _get_nc():
    if "nc" not in _CACHE:
        _CACHE["nc"] = _build()
    return _CACHE["nc"]


def _run(inputs, trace=False):
    nc = _get_nc()
    shared = _prep_shared(inputs)
    X = inputs["X"]
    in_maps = []
    for b in range(N_CORES):
        m = dict(shared)
        m["xt"] = np.ascontiguousarray(X[b].T).astype(BF)
        in_maps.append(m)
    if trace:
        _install_profile_hook()
    res = run_bass_kernel_spmd(nc, in_maps, list(range(N_CORES)), trace=trace)
    out = np.empty((B, S, D), dtype=np.float32)
    for b in range(N_CORES):
        r = res.results[b]
        out[b] = (
            np.asarray(r["out"]).astype(np.float32)
            + np.asarray(r["out2"]).astype(np.float32)
        ).T
    return out, res


def _install_profile_hook():
    import antenv

    if "antenv.axon_hooks" not in sys.modules:
        mod = types.ModuleType("antenv.axon_hooks")
        mod._hook = None
        mod.set_axon_ntff_profile_hook = lambda h: setattr(mod, "_hook", h)
        mod.get_axon_ntff_profile_hook = lambda: mod._hook
        sys.modules["antenv.axon_hooks"] = mod
        antenv.axon_hooks = mod
    hooks = sys.modules["antenv.axon_hooks"]
    if hooks.get_axon_ntff_profile_hook() is None:
        from trn_agent_boot.trn_boot import _ntff_profile_via_ctypes

        hooks.set_axon_ntff_profile_hook(
            _ntff_profile_via_ctypes("/opt/axon/libaxon_pjrt.so")
        )
    import concourse.bass_utils as bass_utils

    bass_utils.upload_artifacts = lambda tmpdir: f"local:{tmpdir}"


def kernel(**inputs) -> np.ndarray:
    out, _ = _run(inputs, trace=False)
    return out


# revision 27
# speedup vs baseline: 1.1309x; 1.0777x over previous
"""Trainium2 Bass kernel for nn_AdditiveAttention (B=8, S=4096, D=1024, H=16).

Sharding: pure data-parallel over batch — 8 NeuronCores, one batch element
per core, weights replicated. No collectives.

Per-core layout: everything transposed (d on partitions, s on free).

v3 structure:
  - Q phase is s-quarter-outer: each 2.1 MB quarter-column of xt is
    consumed as its DMAs land, so the PE starts ~12us into the kernel and
    streams from there. Warm-up matmuls bridge the DMA window (HAM
    un-throttle + ACT exp-table load).
  - All per-chunk ops are 1024-wide (two 512 psum banks per tile) to
    amortize the ~300ns fixed per-op engine overhead.
  - Pooling is incremental: ACT Exp writes accum_out Z-partials, a fused
    DVE scalar_tensor_tensor(mult, mult, accum_out) does the numerator.
    K-phase psum evacuations ride DVE (tensor_scalar_add) to keep ACT
    at the exp-only load.
  - The elementwise V gate u = (v+bv)*gk and the r-projection fold into
    the weights on-device: Wcomb = Wv @ diag(gk_h) @ Wr per head (64
    N=128 fp8 matmuls from Wv^T, built per head-pair block as that
    block's gk finalizes inside the K phase), so the V phase is a single
    streaming fp8 DoubleRow GEMM producing the rt output directly.
  - Outputs are bf16 (q-residual and rt part summed on host in f32).
"""

import sys
import types

import numpy as np
import ml_dtypes

from contextlib import ExitStack

import concourse.bass as bass
import concourse.tile as tile
from concourse import bacc, mybir
from concourse.bass_utils import run_bass_kernel_spmd

B, S, D, H, HD = 8, 4096, 1024, 16, 64
P = 128          # partitions
T = D // P       # 8 d-tiles
NC_ = 512        # psum bank free size
NP = 1024        # paired op width
NG = S // NP     # 4 quarter/pair groups
N_CORES = 8
BF16 = mybir.dt.bfloat16
FP8 = mybir.dt.float8e4
F32 = mybir.dt.float32
W8SCALE = 64.0   # host scales Wk/WvT/Wr by this into e4m3 normal range
GK8 = 256.0      # device scale for gk -> fp8 operand
WCE = 2.0 ** -4  # psWC -> wcomb8 evacuation scale
VDE = 2.0 ** -18 # V-phase psum -> output scale (undoes 64*64*256*WCE)
BF = ml_dtypes.bfloat16
F8 = ml_dtypes.float8_e4m3

_CACHE = {}


def _build():
    nc = bacc.Bacc(
        "TRN2", target_bir_lowering=False, debug=False, num_devices=N_CORES
    )
    xt_ext = nc.declare_dram_parameter("xt", [D, S], BF16, isOutput=False)
    wq_ext = nc.declare_dram_parameter("wq", [D, D], BF16, isOutput=False)
    wk_ext = nc.declare_dram_parameter("wk", [D, D], FP8, isOutput=False)
    wvt_ext = nc.declare_dram_parameter("wvt", [D, D], FP8, isOutput=False)
    bq_ext = nc.declare_dram_parameter("bq", [P, T], F32, isOutput=False)
    bk_ext = nc.declare_dram_parameter("bk", [P, T], F32, isOutput=False)
    bv_ext = nc.declare_dram_parameter("bv", [P, T], F32, isOutput=False)
    wql_ext = nc.declare_dram_parameter("wqlrep", [P, P], BF16, isOutput=False)
    wkl_ext = nc.declare_dram_parameter("wklrep", [P, P], BF16, isOutput=False)
    wrr_ext = nc.declare_dram_parameter("wrr", [P, P], BF16, isOutput=False)
    wrr8_ext = nc.declare_dram_parameter("wrr8", [P, P], FP8, isOutput=False)
    br_ext = nc.declare_dram_parameter("br", [P, 1], F32, isOutput=False)
    out_ext = nc.declare_dram_parameter("out", [D, S], BF16, isOutput=True)
    out2_ext = nc.declare_dram_parameter("out2", [D, S], BF16, isOutput=True)

    AX = mybir.AxisListType.X
    ALU = mybir.AluOpType
    AF = mybir.ActivationFunctionType

    with tile.TileContext(nc) as tc, ExitStack() as ctx:
        singles = ctx.enter_context(tc.tile_pool(name="singles", bufs=1))
        psum = ctx.enter_context(tc.tile_pool(name="psum", bufs=2, space="PSUM"))
        pslg = ctx.enter_context(tc.tile_pool(name="pslg", bufs=2, space="PSUM"))
        ch_pool = ctx.enter_context(tc.tile_pool(name="chpool", bufs=7))
        e_pool = ctx.enter_context(tc.tile_pool(name="epool", bufs=5))
        eff_pool = ctx.enter_context(tc.tile_pool(name="eff", bufs=2))
        small_pool = ctx.enter_context(tc.tile_pool(name="small", bufs=4))

        # ---- resident tiles ----
        xt_sb = singles.tile([P, T, S], BF16, name="xt", tag="xt")
        xt8_sb = singles.tile([P, T, S], FP8, name="xt8", tag="xt8")
        wq_sb = singles.tile([P, T, D], BF16, name="wq", tag="wq")
        wk_sb = singles.tile([P, T, D], FP8, name="wk", tag="wk")
        wvt_sb = singles.tile([P, T, D], FP8, name="wvt", tag="wvt")
        wcomb_sb = singles.tile([P, T, D], FP8, name="wcomb", tag="wcomb")
        wqlrep = singles.tile([P, P], BF16, name="wqlrep", tag="wqlrep")
        wklrep = singles.tile([P, P], BF16, name="wklrep", tag="wklrep")
        wrr = singles.tile([P, P], BF16, name="wrr", tag="wrr")
        wrr8 = singles.tile([P, P], FP8, name="wrr8", tag="wrr8")
        bq_sb = singles.tile([P, T], F32, name="bq", tag="bq")
        bk_sb = singles.tile([P, T], F32, name="bk", tag="bk")
        bk16_sb = singles.tile([P, T], BF16, name="bk16", tag="bk16")
        cb_sb = singles.tile([P, T], F32, name="cb", tag="cb")
        bv_sb = singles.tile([P, T], F32, name="bv", tag="bv")
        bvg_sb = singles.tile([P, T], BF16, name="bvg", tag="bvg")
        br_sb = singles.tile([P, 1], F32, name="br", tag="br")
        bias2_sb = singles.tile([P, T], F32, name="bias2", tag="bias2")
        zq_sb = singles.tile([P, T, NG], F32, name="zq", tag="zq")
        gqp_sb = singles.tile([P, T, NG], F32, name="gqp", tag="gqp")
        zk_sb = singles.tile([P, T, NG], F32, name="zk", tag="zk")
        gkp_sb = singles.tile([P, T, NG], F32, name="gkp", tag="gkp")
        gq_all = singles.tile([P, T], F32, name="gq", tag="gq")
        gk_all = singles.tile([P, T], F32, name="gk", tag="gk")
        gk8c = singles.tile([P, 1], F32, name="gk8c", tag="gk8c")
        effk_sb = singles.tile([P, T, P], BF16, name="effk", tag="effk")
        scratch = singles.tile([P, NC_], BF16, name="scr", tag="scr")
        scr_e = singles.tile([P, NC_], BF16, name="scre", tag="scre")

        # ---- DMA issue ----
        # sync queue: small weights, then wq in t-major blocks (each block
        # unlocks one output tile's GEMMs — block 0 lands ~5us).
        nc.sync.dma_start(wqlrep[:], wql_ext.ap())
        nc.sync.dma_start(wklrep[:], wkl_ext.ap())
        nc.sync.dma_start(wrr[:], wrr_ext.ap())
        nc.sync.dma_start(wrr8[:], wrr8_ext.ap())
        nc.sync.dma_start(bq_sb[:], bq_ext.ap())
        nc.sync.dma_start(bk_sb[:], bk_ext.ap())
        nc.sync.dma_start(bv_sb[:], bv_ext.ap())
        nc.sync.dma_start(br_sb[:], br_ext.ap())
        nc.sync.dma_start(wq_sb[:, 0, :], wq_ext.ap()[0:P, :])
        # quarter 0 is split across both queues so the first GEMMs unblock
        # as early as possible; wq t-blocks 1..7 follow on sync.
        for k in range(0, T, 2):
            nc.sync.dma_start(
                xt_sb[:, k, 0:NP], xt_ext.ap()[k * P : (k + 1) * P, 0:NP]
            )
        for k in range(1, T, 2):
            nc.gpsimd.dma_start(
                xt_sb[:, k, 0:NP], xt_ext.ap()[k * P : (k + 1) * P, 0:NP]
            )
        for t in range(1, T):
            nc.sync.dma_start(wq_sb[:, t, :], wq_ext.ap()[t * P : (t + 1) * P, :])
        # gpsimd queue: remaining xt quarters (landing in consumption
        # order), then the K/V-phase weights. xt8 is derived on-device.
        for g in range(1, NG):
            sl = slice(g * NP, (g + 1) * NP)
            for k in range(T):
                nc.gpsimd.dma_start(
                    xt_sb[:, k, sl], xt_ext.ap()[k * P : (k + 1) * P, sl]
                )
        for k in range(T):
            rsl = slice(k * P, (k + 1) * P)
            nc.gpsimd.dma_start(wk_sb[:, k, :], wk_ext.ap()[rsl, :])
        for k in range(T):
            rsl = slice(k * P, (k + 1) * P)
            nc.gpsimd.dma_start(wvt_sb[:, k, :], wvt_ext.ap()[rsl, :])

        # ---- warm-up during the DMA window: HAM un-throttle + exp table ----
        nc.vector.memset(scratch[:], 0.0)
        warm_ps = pslg.tile([P, NP], F32, name="plg", tag="plg")
        for _ in range(18):
            nc.tensor.matmul(warm_ps[:, 0:NC_], scratch[:, 0:P], scratch[:],
                             start=True, stop=True)
        nc.scalar.activation(scr_e[:], warm_ps[:, 0:NC_], AF.Exp,
                             bias=0.0, scale=1.0)
        nc.vector.tensor_copy(bk16_sb[:], bk_sb[:])
        for _ in range(20):
            nc.tensor.matmul(
                warm_ps[:, 0:NC_], wq_sb[:, 0, 0:P],
                wq_sb[:, 0, 0:NC_], start=True, stop=True,
            )

        # ================= Q phase: s-quarter outer =================
        pend = None  # deferred logit matmul for the previous pair

        def emit_logit_q(t, g, qtc):
            plg = pslg.tile([P, NP], F32, name="plg", tag="plg")
            nc.tensor.matmul(plg[:, 0:NC_], wqlrep[:], qtc[:, 0:NC_],
                             start=True, stop=True)
            nc.tensor.matmul(plg[:, NC_:NP], wqlrep[:], qtc[:, NC_:NP],
                             start=True, stop=True)
            ec = e_pool.tile([P, NP], BF16, name="ec", tag="ec")
            nc.scalar.activation(
                ec[:], plg[:], AF.Exp, bias=0.0, scale=1.0,
                accum_out=zq_sb[:, t, g : g + 1],
            )
            nc.vector.scalar_tensor_tensor(
                ec[:], ec[:], 1.0, qtc[:], ALU.mult, ALU.mult,
                accum_out=gqp_sb[:, t, g : g + 1],
            )

        def finalize_q(t):
            ztot = small_pool.tile([P, 1], F32, name="ztot", tag="ztot")
            nc.vector.reduce_sum(ztot, zq_sb[:, t, :], axis=AX)
            recip = small_pool.tile([P, 1], F32, name="recip", tag="recip")
            nc.vector.reciprocal(recip, ztot)
            graw = small_pool.tile([P, 1], F32, name="graw", tag="graw")
            nc.vector.reduce_sum(graw, gqp_sb[:, t, :], axis=AX)
            nc.vector.tensor_mul(gq_all[:, t : t + 1], graw, recip)
            # eff_kl for the K phase, built as soon as gq[t] exists
            nc.vector.tensor_scalar_mul(
                effk_sb[:, t, :], wklrep[:], gq_all[:, t : t + 1]
            )

        for g in range(NG):
            sl = slice(g * NP, (g + 1) * NP)
            for t in range(T):
                pch = psum.tile([P, NP], F32, name="pch", tag="pch")
                for h in range(2):
                    hsl = slice(g * NP + h * NC_, g * NP + (h + 1) * NC_)
                    psl = slice(h * NC_, (h + 1) * NC_)
                    for k in range(T):
                        nc.tensor.matmul(
                            pch[:, psl],
                            wq_sb[:, t, k * P : (k + 1) * P],
                            xt_sb[:, k, hsl],
                            start=(k == 0),
                            stop=(k == T - 1),
                        )
                qtc = ch_pool.tile([P, NP], BF16, name="qtc", tag="qtc")
                nc.scalar.activation(
                    qtc[:], pch[:], AF.Identity, bias=bq_sb[:, t : t + 1],
                    scale=1.0,
                )
                nc.sync.dma_start(out_ext.ap()[t * P : (t + 1) * P, sl], qtc[:])
                if pend is not None:
                    emit_logit_q(*pend)
                pend = (t, g, qtc)
                # derive the fp8 copy of xt on spare DVE capacity. One
                # k-tile per iteration through quarter 2; quarter 3's
                # conversions are front-loaded so the phase tail keeps DVE
                # free for the finalizers and the K-phase rampup.
                if g < 3:
                    c = g * T + t
                    nc.vector.tensor_copy(
                        xt8_sb[:, c % T, (c // T) * NP : (c // T + 1) * NP],
                        xt_sb[:, c % T, (c // T) * NP : (c // T + 1) * NP],
                    )
                elif t < 4:
                    for c in (24 + 2 * t, 25 + 2 * t):
                        nc.vector.tensor_copy(
                            xt8_sb[:, c % T, (c // T) * NP : (c // T + 1) * NP],
                            xt_sb[:, c % T, (c // T) * NP : (c // T + 1) * NP],
                        )
                if g == NG - 1 and t > 0:
                    finalize_q(t - 1)
        emit_logit_q(*pend)
        finalize_q(T - 1)

        # ================= K phase: tile outer =================
        def emit_logit_k(t, g, pc):
            # pc holds the RAW 64*k chunk; the bk part of the logit is the
            # per-column constant cb[t] (folded into the exp bias), and the
            # bk part of the pooling numerator folds into the stt op0.
            plg = pslg.tile([P, NP], F32, name="plg", tag="plg")
            nc.tensor.matmul(plg[:, 0:NC_], effk_sb[:, t, :], pc[:, 0:NC_],
                             start=True, stop=True)
            nc.tensor.matmul(plg[:, NC_:NP], effk_sb[:, t, :], pc[:, NC_:NP],
                             start=True, stop=True)
            ec = e_pool.tile([P, NP], BF16, name="ec", tag="ec")
            nc.scalar.activation(
                ec[:], plg[:], AF.Exp, bias=cb_sb[:, t : t + 1], scale=1.0,
                accum_out=zk_sb[:, t, g : g + 1],
            )
            nc.vector.scalar_tensor_tensor(
                ec[:], pc[:], bk_sb[:, t : t + 1], ec[:], ALU.add, ALU.mult,
                accum_out=gkp_sb[:, t, g : g + 1],
            )

        def finalize_k(t):
            """gk_all[:, t] = gq * (sum_g gkp / (64 * sum_g zk)) — true gk."""
            ztot = small_pool.tile([P, 1], F32, name="ztot", tag="ztot")
            nc.vector.reduce_sum(ztot, zk_sb[:, t, :], axis=AX)
            nc.vector.tensor_scalar_mul(ztot, ztot, W8SCALE)
            recip = small_pool.tile([P, 1], F32, name="recip", tag="recip")
            nc.vector.reciprocal(recip, ztot)
            graw = small_pool.tile([P, 1], F32, name="graw", tag="graw")
            nc.vector.reduce_sum(graw, gkp_sb[:, t, :], axis=AX)
            nc.vector.tensor_mul(graw, graw, recip)
            nc.vector.tensor_mul(gk_all[:, t : t + 1], graw, gq_all[:, t : t + 1])

        def build_wcomb(j):
            """wcomb8[:, :, j-cols] = 2^18 * Wv @ diag(gk_j) @ Wr, block j."""
            nc.vector.tensor_scalar_mul(gk8c[:], gk_all[:, j : j + 1], GK8)
            gkwr = eff_pool.tile([P, P], FP8, name="gkwr", tag="gkwr")
            nc.vector.tensor_scalar_mul(gkwr[:], wrr8[:], gk8c[:, 0:1])
            pw = pslg.tile([P, NP], F32, name="plg", tag="plg")
            for tt in range(T):
                nc.tensor.matmul(
                    pw[:, tt * P : (tt + 1) * P],
                    wvt_sb[:, j, tt * P : (tt + 1) * P], gkwr[:],
                    start=True, stop=True,
                )
            nc.scalar.activation(
                wcomb_sb[:, :, j * P : (j + 1) * P], pw[:],
                AF.Identity, bias=0.0, scale=WCE,
            )
            # bias2_j = Wr^T (bv*gk)_j + br    (true scale)
            nc.vector.tensor_mul(
                bvg_sb[:, j : j + 1], bv_sb[:, j : j + 1], gk_all[:, j : j + 1]
            )
            pb = pslg.tile([P, NP], F32, name="plg", tag="plg")
            nc.tensor.matmul(
                pb[:, 0:1], wrr[:], bvg_sb[:, j : j + 1], start=True, stop=True
            )
            nc.scalar.activation(
                bias2_sb[:, j : j + 1], pb[:, 0:1], AF.Identity,
                bias=br_sb[:, 0:1], scale=1.0,
            )

        def emit_v_pair(j, g):
            """One V-phase GEMM pair for tile j (gate folded into wcomb)."""
            sl = slice(g * NP, (g + 1) * NP)
            pv = psum.tile([P, NP], F32, name="pch", tag="pch")
            for h in range(2):
                hsl = slice(g * NP + h * NC_, g * NP + (h + 1) * NC_)
                psl = slice(h * NC_, (h + 1) * NC_)
                for k in range(0, T, 2):
                    nc.tensor.matmul(
                        pv[:, psl],
                        wcomb_sb[:, k : k + 2, j * P : (j + 1) * P],
                        xt8_sb[:, k : k + 2, hsl],
                        start=(k == 0),
                        stop=(k == T - 2),
                        perf_mode=mybir.MatmulPerfMode.DoubleRow,
                    )
            stg = ch_pool.tile([P, NP], BF16, name="qtc", tag="qtc")
            nc.scalar.activation(
                stg[:], pv[:], AF.Identity,
                bias=bias2_sb[:, j : j + 1], scale=VDE,
            )
            nc.sync.dma_start(out2_ext.ap()[j * P : (j + 1) * P, sl], stg[:])

        pendq = []
        for t in range(T):
            # cb[t] = effk[t]^T bk64 — the constant column of the beta logit
            cbp = pslg.tile([P, NP], F32, name="plg", tag="plg")
            nc.tensor.matmul(cbp[:, 0:1], effk_sb[:, t, :],
                             bk16_sb[:, t : t + 1], start=True, stop=True)
            nc.scalar.activation(cb_sb[:, t : t + 1], cbp[:, 0:1],
                                 AF.Identity, bias=0.0, scale=1.0)
            for g in range(NG):
                pch = psum.tile([P, NP], F32, name="pch", tag="pch")
                for h in range(2):
                    hsl = slice(g * NP + h * NC_, g * NP + (h + 1) * NC_)
                    psl = slice(h * NC_, (h + 1) * NC_)
                    for k in range(0, T, 2):
                        nc.tensor.matmul(
                            pch[:, psl],
                            wk_sb[:, k : k + 2, t * P : (t + 1) * P],
                            xt8_sb[:, k : k + 2, hsl],
                            start=(k == 0),
                            stop=(k == T - 2),
                            perf_mode=mybir.MatmulPerfMode.DoubleRow,
                        )
                pc = ch_pool.tile([P, NP], BF16, name="qtc", tag="qtc")
                if g % 2 == 0:
                    nc.scalar.activation(pc[:], pch[:], AF.Identity,
                                         bias=0.0, scale=1.0)
                else:
                    nc.vector.tensor_copy(pc[:], pch[:])
                pendq.append((t, g, pc))
                if len(pendq) > 2:
                    emit_logit_k(*pendq.pop(0))
                if t >= 2:
                    # V-phase pairs for tile t-2 interleave here: their
                    # matmuls depend only on wcomb (built during tile t-1)
                    # and xt8, so they fill the K pipeline's stall windows.
                    emit_v_pair(t - 2, g)
                if t > 0:
                    if g == 2:
                        finalize_k(t - 1)
                    elif g == 3:
                        build_wcomb(t - 1)

        # ---- V tail: tiles T-2 and T-1, with the last block's logits,
        # finalize and Wcomb build threaded between tile T-2's pairs ----
        for g in range(NG):
            emit_v_pair(T - 2, g)
            if g == 0:
                while pendq:
                    emit_logit_k(*pendq.pop(0))
            elif g == 1:
                finalize_k(T - 1)
            elif g == 2:
                build_wcomb(T - 1)
        for g in range(NG):
            emit_v_pair(T - 1, g)

    nc.compile()
    return nc


def _prep_shared(inputs):
    """Host-side prep of the replicated (weight) arrays."""
    sc = 0.125  # 1/sqrt(HD)

    def rep_logit(w, scale):
        m = np.zeros((P, P), dtype=np.float32)
        ws = w.astype(np.float32) * scale
        m[:HD, :HD] = ws[:, None]
        m[HD:, HD:] = ws[:, None]
        return m.astype(BF)

    def bias_pp(b):
        return np.ascontiguousarray(b.astype(np.float32).reshape(T, P).T)

    wrrf = np.zeros((P, P), dtype=np.float32)
    wr = inputs["Wr"].astype(np.float32)
    wrrf[:HD, :HD] = wr
    wrrf[HD:, HD:] = wr

    wq_tmaj = (
        inputs["Wq"].astype(np.float32)
        .reshape(T, P, T, P).transpose(2, 1, 0, 3).reshape(D, D)
    )
    return {
        "wq": np.ascontiguousarray(wq_tmaj.astype(BF)),
        "wk": np.ascontiguousarray(
            (inputs["Wk"].astype(np.float32) * W8SCALE).astype(F8)
        ),
        "wvt": np.ascontiguousarray(
            (inputs["Wv"].astype(np.float32).T * W8SCALE).astype(F8)
        ),
        "bq": bias_pp(inputs["bq"]),
        "bk": bias_pp(inputs["bk"]) * np.float32(W8SCALE),
        "bv": bias_pp(inputs["bv"]),
        "wqlrep": rep_logit(inputs["wql"], sc),
        "wklrep": rep_logit(inputs["wkl"], sc / W8SCALE),
        "wrr": wrrf.astype(BF),
        "wrr8": (wrrf * W8SCALE).astype(F8),
        "br": np.ascontiguousarray(
            np.tile(inputs["br"].astype(np.float32), 2).reshape(P, 1)
        ),
    }


def _get_nc():
    if "nc" not in _CACHE:
        _CACHE["nc"] = _build()
    return _CACHE["nc"]


def _run(inputs, trace=False):
    nc = _get_nc()
    shared = _prep_shared(inputs)
    X = inputs["X"]
    in_maps = []
    for b in range(N_CORES):
        m = dict(shared)
        m["xt"] = np.ascontiguousarray(X[b].T).astype(BF)
        in_maps.append(m)
    if trace:
        _install_profile_hook()
    res = run_bass_kernel_spmd(nc, in_maps, list(range(N_CORES)), trace=trace)
    out = np.empty((B, S, D), dtype=np.float32)
    for b in range(N_CORES):
        r = res.results[b]
        out[b] = (
            np.asarray(r["out"]).astype(np.float32)
            + np.asarray(r["out2"]).astype(np.float32)
        ).T
    return out, res


def _install_profile_hook():
    import antenv

    if "antenv.axon_hooks" not in sys.modules:
        mod = types.ModuleType("antenv.axon_hooks")
        mod._hook = None
        mod.set_axon_ntff_profile_hook = lambda h: setattr(mod, "_hook", h)
        mod.get_axon_ntff_profile_hook = lambda: mod._hook
        sys.modules["antenv.axon_hooks"] = mod
        antenv.axon_hooks = mod
    hooks = sys.modules["antenv.axon_hooks"]
    if hooks.get_axon_ntff_profile_hook() is None:
        from trn_agent_boot.trn_boot import _ntff_profile_via_ctypes

        hooks.set_axon_ntff_profile_hook(
            _ntff_profile_via_ctypes("/opt/axon/libaxon_pjrt.so")
        )
    import concourse.bass_utils as bass_utils

    bass_utils.upload_artifacts = lambda tmpdir: f"local:{tmpdir}"


def kernel(**inputs) -> np.ndarray:
    out, _ = _run(inputs, trace=False)
    return out
